# revision 7
# baseline (speedup 1.0000x reference)
"""GAT attention layer (EEGGraphAttentionLayer) for Trainium2, 8 NeuronCores.

reference math:
    Wh = h @ w                         # (8192, 64)
    e  = leaky_relu((Wh@a_src) + (Wh@a_dst).T, slope=0.2)   # (8192, 8192)
    att = where(adj > 0, e, -1e12)
    out = softmax(att, axis=1)

Sharding: rows of adj/out across 8 cores (1024 rows each); row softmax is
core-local. Each core recomputes the column-score vector s2 = h @ (w@a_dst)
(an N-vector) from the full h. h and w are host-transposed and h/adj
precision-reduced (bf16 / fp8e5 -- pure dtype casts); output bf16, upcast
on host (exact).

Math restructure (key to speed): with per-row shift M_i = s1_i + K
(K ~= max_j s2_j), max distributes over the shift so
    lrelu(z) - M = max(z - M, 0.2*z - M)
                 = max(m, 0.2*m - 0.8*C*M)   with m = C*(s2_j - K) row-free!
The fused per-element op is only FOUR ALU ops:
    u = max(min(bc2', adj)*0.2 + c1[p], min(bc2', adj))
with bc2'[j] = C*(s2_j - K) (bf16, shared by all row tiles) and
c1[p] = -0.8*C*(s1_p + K) per-partition. Masked entries ride the C-scale
min trick: |C*(s2-K)| <= ~5e-11 << 2^-16 (smallest positive fp8), so min
selects adj (<= -2^-16) when masked and exp(2^40 * 0.2*adj) underflows to
exactly 0. The shift K comes from one 512-column PSUM chunk (cheap, and any
K near max(s2) only affects rounding centering, not correctness).

Engine mapping per row tile [128, 8192]:
    fused op   DVE  custom GAT_MLRS_ANT with a hand-authored 2x_1p uop
               program (dual 4-op chains in the 8 ALU blocks) -> 2 elem/cyc.
               adj is upconverted fp8e5->bf16 *inside the load DMA* (SWDGE
               cast) so both tensor operands are 2-byte = 2x eligible,
               while HBM traffic stays 1 B/elem.
    exp+rowsum ACT  one Exp pass, scale=2^40, accum_out (1 elem/cyc)
    out*(1/S)  DVE  tensor_scalar, bf16 4x mode
HBM per core: 8 MB adj(fp8) + 16 MB out(bf16) + 2.25 MB h.
"""
import os
import sys

for _p in (
    "/opt/trn_rl_repo",
    "/root/.axon_site/_ro/trn_rl_repo",
):
    if os.path.isdir(_p) and _p not in sys.path:
        sys.path.append(_p)

import numpy as np
import ml_dtypes


def _install_profile_shim():
    """bass_utils' trace path imports antenv.axon_hooks, which this image
    lacks. Provide it (with the ctypes hook into libaxon if available) so a
    BASS_TRACE=1 run profiles instead of crashing. No-op on any failure."""
    import contextlib
    import ctypes
    import types

    if "antenv.axon_hooks" in sys.modules:
        return
    try:
        import antenv
    except ImportError:
        return

    def _make_hook(so_path):
        try:
            lib = ctypes.CDLL(so_path)
        except OSError:
            return None
        if not hasattr(lib, "axon_start_nrt_profile"):
            return None
        lib.axon_start_nrt_profile.argtypes = [
            ctypes.POINTER(ctypes.c_int64),
            ctypes.c_size_t,
        ]
        lib.axon_start_nrt_profile.restype = ctypes.c_int64
        lib.axon_stop_nrt_profile.argtypes = [ctypes.c_char_p]
        lib.axon_stop_nrt_profile.restype = ctypes.c_int64

        @contextlib.contextmanager
        def _hook(output_dir, device_ids):
            import jax

            jax.devices()
            if device_ids:
                ids = (ctypes.c_int64 * len(device_ids))(*device_ids)
                rc = lib.axon_start_nrt_profile(ids, len(device_ids))
            else:
                rc = lib.axon_start_nrt_profile(None, 0)
            if rc != 0:
                raise RuntimeError(f"axon_start_nrt_profile rc={rc}")
            try:
                yield
            finally:
                n = lib.axon_stop_nrt_profile(str(output_dir).encode())
                print(f"profile: {n} file(s) -> {output_dir}", file=sys.stderr)

        return _hook

    hook = [_make_hook("/opt/axon/libaxon_pjrt.so")]
    mod = types.ModuleType("antenv.axon_hooks")
    mod.set_axon_ntff_profile_hook = lambda h: hook.__setitem__(0, h)
    mod.get_axon_ntff_profile_hook = lambda: hook[0]
    sys.modules["antenv.axon_hooks"] = mod
    antenv.axon_hooks = mod


try:
    _install_profile_shim()
except Exception:
    pass

import concourse.bacc as bacc
import concourse.tile as tile
import concourse.bass as bass
from concourse import mybir
from concourse.bass_utils import run_bass_kernel_spmd

N, F_IN, F_OUT = 8192, 128, 64
NCORES = 8
R = N // NCORES          # rows per core (1024)
P = 128                  # SBUF partitions
RT = R // P              # row tiles per core (8)
C = 2.0 ** -40           # exact scale-down of scores
CI = 2.0 ** 40
ALPHA = 0.2              # leaky relu negative slope
F32 = mybir.dt.float32
BF16 = mybir.dt.bfloat16
FP8 = mybir.dt.float8e5
AF = mybir.ActivationFunctionType
ALU = mybir.AluOpType

_CACHED_NC = None
LAST_RESULT = None       # BassKernelResults of the most recent run (for tests)


def _register_gat_op():
    """Register the fused mask+lrelu custom DVE op (idempotent).

    out = max(min(in0, in1)*imm2 + s1, min(in0, in1))

    Base (1x) program comes from lower(); a hand-authored dual-chain
    2x_1p program is attached at perf slot +1 so the engine processes two
    bf16 elements per cycle when operands are packed 16-bit step-1 SBUF.
    """
    import concourse.dve_ops as dve_ops
    from concourse.dve_spec import Spec, Src0, Src1, C1, C2, maxx, minn, lower
    from concourse.dve_uop import (
        DveOpSpec,
        UopConfig,
        UopDpConfig,
        InpSel,
        OutSel,
        OutPath,
        AluInp,
        AluOp,
        DelayInp,
        Trigger,
    )

    name = "GAT_MLRS_ANT"
    for op in dve_ops.OPS:
        if op.name == name:
            return op

    def _ref(in0, in1, s0, s1, imm2):
        m = np.minimum(in0.astype(np.float32), in1.astype(np.float32))
        return np.maximum(m * imm2 + s1, m).astype(np.float32)

    _m = minn(Src0, Src1)
    spec = Spec(body=maxx(_m * C2 + C1, _m), reference=_ref)

    # ---- hand-authored 2x_1p uop: two parallel 4-op chains ------------
    # chain A (even elems): b0 MIN, b2 MUL, b4 ADD, b6 MAX
    # chain B (odd  elems): b1 MIN, b3 MUL, b5 ADD, b7 MAX
    # lanes: d0=SRC_1->mB, d1=SRC_0_HI->tA->qA->uA, d2=SRC_1_HI->tB->qB,
    #        d3=CONST_1(c1), d4=CONST_2(imm2), d5=mA
    def pd(k):
        return AluInp(int(AluInp.PREV_DELAY_0) + k)

    u2 = UopConfig()
    u2.enable_input(InpSel.SRC_0, 0)       # ALU path into block 0
    u2.enable_input(InpSel.SRC_1, 1)       # -> delay lane 0
    u2.enable_input(InpSel.SRC_0_HI, 2)    # -> delay lane 1
    u2.enable_input(InpSel.SRC_1_HI, 3)    # -> delay lane 2
    u2.enable_input(InpSel.CONST_1, 4)     # -> delay lane 3  (s1 slot)
    u2.enable_input(InpSel.CONST_2, 5)     # -> delay lane 4  (imm2 slot)
    dp = u2.datapath_config
    for b in dp:
        b.pass_through_delay(0, 1, 2, 3, 4, 5)
    dp[0].enable_alu(AluOp.MIN, AluInp.PREV_ALU_OUT, pd(0))        # mA
    dp[1].enable_alu(AluOp.MIN, pd(1), pd(2))                      # mB
    dp[1].enable_delay_from_src(DelayInp.PREV_ALU_OUT, 5)          # d5 <- mA
    dp[2].enable_alu(AluOp.MULTIPLY, pd(5), pd(4))                 # tA
    dp[2].enable_delay_from_src(DelayInp.PREV_ALU_OUT, 0)          # d0 <- mB
    dp[3].enable_alu(AluOp.MULTIPLY, pd(0), pd(4))                 # tB
    dp[3].enable_delay_from_src(DelayInp.PREV_ALU_OUT, 1)          # d1 <- tA
    dp[4].enable_alu(AluOp.ADD, pd(1), pd(3))                      # qA
    dp[4].enable_delay_from_src(DelayInp.PREV_ALU_OUT, 2)          # d2 <- tB
    dp[5].enable_alu(AluOp.ADD, pd(2), pd(3))                      # qB
    dp[5].enable_delay_from_src(DelayInp.PREV_ALU_OUT, 1)          # d1 <- qA
    dp[6].enable_alu(AluOp.MAX, pd(1), pd(5))                      # uA
    dp[6].enable_delay_from_src(DelayInp.PREV_ALU_OUT, 2)          # d2 <- qB
    dp[7].enable_alu(AluOp.MAX, pd(2), pd(0))                      # uB
    dp[7].enable_delay_from_src(DelayInp.PREV_ALU_OUT, 1)          # d1 <- uA
    u2.enable_output(OutSel.DELAY_1, OutPath.WR0_LO)               # uA (even)
    u2.enable_output(OutSel.ALU_OUT, OutPath.WR0_HI)               # uB (odd)
    u2.require_inp0 = 1
    u2.require_inp1 = 1
    u2.trigger = (Trigger.SRC_TENSOR_DONE, Trigger.NONE, Trigger.NONE)

    row = 1 + len(dve_ops.OPS)
    ver = "v3"
    s = DveOpSpec(
        name=name,
        opcode=row,
        uops=lower(spec, ver=ver),
        uops_2x=[u2],
        perf_max=1,
        rd1_en=True,
    )
    op = dve_ops.DveOp(name, spec, subdim=False, uops_sha={ver: s.sha(ver)})
    dve_ops.OPS.append(op)
    dve_ops._SUB_OPCODE_FOR_NAME[name] = row
    dve_ops.CUSTOM_DVE_SPECS[name] = op.spec
    dve_ops._COMPILE_CACHE[(name, ver)] = s
    return op


GAT_OP = _register_gat_op()


def build_nc():
    nc = bacc.Bacc("TRN2", target_bir_lowering=False)
    hT_d = nc.dram_tensor("hT", [F_IN, N], BF16, kind="ExternalInput")
    hsT_d = nc.dram_tensor("hsT", [F_IN, R], BF16, kind="ExternalInput")
    adj_d = nc.dram_tensor("adj", [R, N], FP8, kind="ExternalInput")
    wT_d = nc.dram_tensor("wT", [F_OUT, F_IN], F32, kind="ExternalInput")
    a_d = nc.dram_tensor("a", [2 * F_OUT, 1], F32, kind="ExternalInput")
    out_d = nc.dram_tensor("out", [R, N], BF16, kind="ExternalOutput")

    with tile.TileContext(nc) as tc:
        with (
            tc.tile_pool(name="persist", bufs=1) as persist,
            tc.tile_pool(name="hTp", bufs=8) as hTp,
            tc.tile_pool(name="psB", bufs=4, space="PSUM") as psB,
            tc.tile_pool(name="psS", bufs=1, space="PSUM") as psS,
            tc.tile_pool(name="adjp8", bufs=2) as adjp8,
            tc.tile_pool(name="adjp", bufs=4) as adjp,
            tc.tile_pool(name="ep", bufs=5) as ep,
            tc.tile_pool(name="small", bufs=6) as small,
        ):
            # --------- setup: tiny params first, then the bulk streams ------
            wT_sb = persist.tile([F_OUT, F_IN], F32)
            nc.scalar.dma_start(out=wT_sb, in_=wT_d[:, :])
            # a2[o, j] = a[j*64 + o]: a_src / a_dst as two columns
            a2 = persist.tile([F_OUT, 2], F32)
            a_t = a_d.tensor if hasattr(a_d, "tensor") else a_d
            nc.scalar.dma_start(
                out=a2, in_=bass.AP(tensor=a_t, offset=0, ap=[[1, F_OUT], [F_OUT, 2]])
            )
            hsT_sb = persist.tile([P, R], BF16)
            nc.sync.dma_start(out=hsT_sb, in_=hsT_d[:, :])
            hTs = []
            for g in range(8):
                hTc = hTp.tile([P, N // 8], BF16, tag="hTc")
                eng = nc.scalar if g % 2 == 0 else nc.sync
                eng.dma_start(
                    out=hTc, in_=hT_d[:, g * (N // 8):(g + 1) * (N // 8)]
                )
                hTs.append(hTc)
            # adj loads, all on the SWDGE ring (FIFO):
            #  - tiles 0-1 ride as raw fp8 (cheap 1 MB loads, land early so
            #    the main loop starts ~16us; their fused op auto-falls back
            #    to the 1x program since in1 is 1-byte).
            #  - tiles 2-7 use the inline fp8e5 -> bf16 upconvert so their
            #    fused op runs 2x. These are gated on the last hT chunk
            #    (tiny dummy write -> WAR dep): the SDMA engines round-robin
            #    packet-wise across queues and the 2MB cast streams would
            #    otherwise starve the small hT loads the BC2 matmuls need.
            NFP8 = 2
            adjts = []
            for t in range(RT):
                if t < NFP8:
                    adjt = adjp8.tile([P, N], FP8, tag="adjt8")
                    nc.gpsimd.dma_start(
                        out=adjt, in_=adj_d[t * P:(t + 1) * P, :]
                    )
                else:
                    adjt = adjp.tile([P, N], BF16, tag="adjt")
                    nc.vector.tensor_scalar(
                        out=adjt[:, 0:1], in0=hTs[7][:, 0:1], scalar1=0.0,
                        scalar2=None, op0=ALU.mult,
                    )
                    nc.gpsimd.dma_start(
                        out=adjt, in_=adj_d[t * P:(t + 1) * P, :]
                    )
                adjts.append(adjt)

            # wa12[:, j] = w @ (a_src if j==0 else a_dst), one K=64 matmul
            ps_wa = psS.tile([P, 2], F32, tag="pswa")
            nc.tensor.matmul(ps_wa, lhsT=wT_sb, rhs=a2, start=True, stop=True)
            wa12 = persist.tile([P, 2], F32)
            nc.scalar.copy(wa12, ps_wa)

            # W2B[f, p] = C * wa2[f]  (stationary matrix for the BC2 matmuls)
            ones = persist.tile([P, P], BF16)
            nc.vector.memset(ones, 1.0)
            w2b = persist.tile([P, P], BF16)
            nc.vector.tensor_scalar(
                out=w2b, in0=ones, scalar1=wa12[:, 1:2], scalar2=C,
                op0=ALU.mult, op1=ALU.mult,
            )
            wa1c = persist.tile([P, 1], BF16)
            nc.vector.tensor_scalar(
                out=wa1c, in0=wa12[:, 0:1], scalar1=C, scalar2=None, op0=ALU.mult
            )

            # s1c[r, t] = C * s1[t*128 + r]  for this core's 8 row tiles
            ps_s1 = psS.tile([P, RT], F32)
            for t in range(RT):
                nc.tensor.matmul(
                    ps_s1[:, t:t + 1], lhsT=hsT_sb[:, t * P:(t + 1) * P],
                    rhs=wa1c, start=True, stop=True,
                )
            s1c = persist.tile([P, RT], F32)
            nc.scalar.copy(s1c, ps_s1)

            # BC2 chunks: psb = C*s2[j] broadcast over partitions.
            # Chunk 0 additionally yields the shift K ~= max_j s2 (a 512-col
            # sample max -- only rounding centering depends on it), then every
            # chunk is copied PSUM -> bc2p with the -K bias applied BEFORE the
            # bf16 rounding (ACT Identity-with-bias; a few ride DVE to
            # balance engine load).
            bc2p = persist.tile([P, N], BF16)
            s2maxc = persist.tile([P, 1], F32)
            sneg = persist.tile([P, 1], F32)
            c1c = persist.tile([P, RT], F32)
            for cg in range(16):
                psb = psB.tile([P, 512], F32, tag="psb")
                nc.tensor.matmul(
                    psb, lhsT=w2b,
                    rhs=hTs[cg // 2][:, (cg % 2) * 512:(cg % 2) * 512 + 512],
                    start=True, stop=True,
                )
                if cg == 0:
                    nc.vector.tensor_reduce(
                        s2maxc, psb, axis=mybir.AxisListType.X, op=ALU.max
                    )
                    nc.vector.tensor_scalar(
                        out=sneg, in0=s2maxc, scalar1=-1.0, scalar2=None,
                        op0=ALU.mult,
                    )
                    # c1[p, t] = -0.8 * (C*s1 + K)
                    nc.vector.tensor_scalar(
                        out=c1c, in0=s1c, scalar1=s2maxc[:, 0:1], scalar2=-0.8,
                        op0=ALU.add, op1=ALU.mult,
                    )
                sl = slice(cg * 512, (cg + 1) * 512)
                # all copies on ACT: the scalar engine is idle during setup
                # (first Exp can only start after fused(0)), so these are
                # free; DVE setup work would delay fused(0).
                nc.scalar.activation(
                    out=bc2p[:, sl], in_=psb, func=AF.Identity,
                    bias=sneg[:, 0:1], scale=1.0,
                )

            # ---------------- main loop over row tiles (sw-pipelined) -------
            # chain per tile:
            #   u = max(min(bc2p, adj)*0.2 + c1, min(bc2p, adj))  (DVE 2x)
            #   p = Exp(2^40 * u), S = rowsum                     (ACT)
            #   out = p * (1/S)                                   (DVE 4x)
            def emit_fused(t):
                et = ep.tile([P, N], BF16, tag="et")
                bi = nc.vector._custom_dve(
                    GAT_OP, out=et, in0=bc2p, in1=adjts[t],
                    s0=0.0, s1=c1c[:, t:t + 1], imm2=ALPHA,
                )
                bi.ins.perf_max = 1
                return et

            LOOKAHEAD = 2
            ets = {t: emit_fused(t) for t in range(min(LOOKAHEAD, RT))}
            for t in range(RT):
                if t + LOOKAHEAD < RT:
                    ets[t + LOOKAHEAD] = emit_fused(t + LOOKAHEAD)
                et = ets.pop(t)
                S = small.tile([P, 1], F32, tag="S")
                nc.scalar.activation(
                    out=et, in_=et, func=AF.Exp,
                    bias=0.0, scale=CI, accum_out=S,
                )
                rs = small.tile([P, 1], F32, tag="rs")
                nc.vector.reciprocal(rs, S)
                # scale + store in halves: each half streams out on its own
                # HWDGE ring as soon as it is scaled, halving the store tail
                # and splitting ring pressure.
                H = N // 2
                for hx in range(2):
                    sl = slice(hx * H, (hx + 1) * H)
                    nc.vector.tensor_scalar(
                        out=et[:, sl], in0=et[:, sl], scalar1=rs[:, 0:1],
                        scalar2=None, op0=ALU.mult,
                    )
                    eng = nc.sync if hx == 0 else nc.scalar
                    eng.dma_start(
                        out=out_d[t * P:(t + 1) * P, sl], in_=et[:, sl]
                    )

    nc.compile()
    return nc


def kernel(h, adj, w, a):
    global _CACHED_NC, LAST_RESULT
    h = np.ascontiguousarray(h, dtype=np.float32)
    adj = np.ascontiguousarray(adj, dtype=np.float32)
    w = np.ascontiguousarray(w, dtype=np.float32)
    a = np.ascontiguousarray(a, dtype=np.float32)

    if _CACHED_NC is None:
        _CACHED_NC = build_nc()
    nc = _CACHED_NC

    hT = np.ascontiguousarray(h.T.astype(ml_dtypes.bfloat16))
    wT = np.ascontiguousarray(w.T)
    in_maps = [
        {
            "hT": hT,
            "hsT": np.ascontiguousarray(hT[:, i * R:(i + 1) * R]),
            "adj": np.ascontiguousarray(
                adj[i * R:(i + 1) * R].astype(ml_dtypes.float8_e5m2)
            ),
            "wT": wT,
            "a": a,
        }
        for i in range(NCORES)
    ]
    res = run_bass_kernel_spmd(nc, in_maps, core_ids=list(range(NCORES)))
    LAST_RESULT = res
    return np.concatenate(
        [r["out"].astype(np.float32) for r in res.results], axis=0
    )


# revision 9
# speedup vs baseline: 1.0978x; 1.0978x over previous
"""GAT attention layer (EEGGraphAttentionLayer) for Trainium2, 8 NeuronCores.

reference math:
    Wh = h @ w                         # (8192, 64)
    e  = leaky_relu((Wh@a_src) + (Wh@a_dst).T, slope=0.2)   # (8192, 8192)
    att = where(adj > 0, e, -1e12)
    out = softmax(att, axis=1)

Sharding: rows of adj/out across 8 cores (1024 rows each); row softmax is
core-local. Each core recomputes the column-score vector s2 = h @ (w@a_dst)
(an N-vector) from the full h. h and w are host-transposed and h/adj
precision-reduced (bf16 / fp8e5 -- pure dtype casts); output bf16, upcast
on host (exact).

Math restructure (key to speed): with per-row shift M_i = s1_i + K
(K ~= max_j s2_j), max distributes over the shift so
    lrelu(z) - M = max(z - M, 0.2*z - M)
                 = max(m, 0.2*m - 0.8*C*M)   with m = C*(s2_j - K) row-free!
The fused per-element op is only FOUR ALU ops:
    u = max(min(bc2', adj)*0.2 + c1[p], min(bc2', adj))
with bc2'[j] = C*(s2_j - K) (bf16, shared by all row tiles) and
c1[p] = -0.8*C*(s1_p + K) per-partition. Masked entries ride the C-scale
min trick: |C*(s2-K)| <= ~5e-11 << 2^-16 (smallest positive fp8), so min
selects adj (<= -2^-16) when masked and exp(2^40 * 0.2*adj) underflows to
exactly 0. The shift K comes from one 512-column PSUM chunk (cheap, and any
K near max(s2) only affects rounding centering, not correctness).

Engine mapping per row tile [128, 8192]:
    fused op   DVE  custom GAT_MLRS_ANT with a hand-authored 2x_1p uop
               program (dual 4-op chains in the 8 ALU blocks) -> 2 elem/cyc.
               adj is upconverted fp8e5->bf16 *inside the load DMA* (SWDGE
               cast) so both tensor operands are 2-byte = 2x eligible,
               while HBM traffic stays 1 B/elem.
    exp+rowsum ACT  one Exp pass, scale=2^40, accum_out (1 elem/cyc)
    out*(1/S)  DVE  tensor_scalar, bf16 4x mode
HBM per core: 8 MB adj(fp8) + 16 MB out(bf16) + 2.25 MB h.
"""
import os
import sys

for _p in (
    "/opt/trn_rl_repo",
    "/root/.axon_site/_ro/trn_rl_repo",
):
    if os.path.isdir(_p) and _p not in sys.path:
        sys.path.append(_p)

import numpy as np
import ml_dtypes


def _install_profile_shim():
    """bass_utils' trace path imports antenv.axon_hooks, which this image
    lacks. Provide it (with the ctypes hook into libaxon if available) so a
    BASS_TRACE=1 run profiles instead of crashing. No-op on any failure."""
    import contextlib
    import ctypes
    import types

    if "antenv.axon_hooks" in sys.modules:
        return
    try:
        import antenv
    except ImportError:
        return

    def _make_hook(so_path):
        try:
            lib = ctypes.CDLL(so_path)
        except OSError:
            return None
        if not hasattr(lib, "axon_start_nrt_profile"):
            return None
        lib.axon_start_nrt_profile.argtypes = [
            ctypes.POINTER(ctypes.c_int64),
            ctypes.c_size_t,
        ]
        lib.axon_start_nrt_profile.restype = ctypes.c_int64
        lib.axon_stop_nrt_profile.argtypes = [ctypes.c_char_p]
        lib.axon_stop_nrt_profile.restype = ctypes.c_int64

        @contextlib.contextmanager
        def _hook(output_dir, device_ids):
            import jax

            jax.devices()
            if device_ids:
                ids = (ctypes.c_int64 * len(device_ids))(*device_ids)
                rc = lib.axon_start_nrt_profile(ids, len(device_ids))
            else:
                rc = lib.axon_start_nrt_profile(None, 0)
            if rc != 0:
                raise RuntimeError(f"axon_start_nrt_profile rc={rc}")
            try:
                yield
            finally:
                n = lib.axon_stop_nrt_profile(str(output_dir).encode())
                print(f"profile: {n} file(s) -> {output_dir}", file=sys.stderr)

        return _hook

    hook = [_make_hook("/opt/axon/libaxon_pjrt.so")]
    mod = types.ModuleType("antenv.axon_hooks")
    mod.set_axon_ntff_profile_hook = lambda h: hook.__setitem__(0, h)
    mod.get_axon_ntff_profile_hook = lambda: hook[0]
    sys.modules["antenv.axon_hooks"] = mod
    antenv.axon_hooks = mod


try:
    _install_profile_shim()
except Exception:
    pass

import concourse.bacc as bacc
import concourse.tile as tile
import concourse.bass as bass
from concourse import mybir
from concourse.bass_utils import run_bass_kernel_spmd

N, F_IN, F_OUT = 8192, 128, 64
NCORES = 8
R = N // NCORES          # rows per core (1024)
P = 128                  # SBUF partitions
RT = R // P              # row tiles per core (8)
C = 2.0 ** -40           # exact scale-down of scores
CI = 2.0 ** 40
ALPHA = 0.2              # leaky relu negative slope
F32 = mybir.dt.float32
BF16 = mybir.dt.bfloat16
FP8 = mybir.dt.float8e5
AF = mybir.ActivationFunctionType
ALU = mybir.AluOpType

_CACHED_NC = None
LAST_RESULT = None       # BassKernelResults of the most recent run (for tests)


def _register_gat_op():
    """Register the fused mask+lrelu custom DVE op (idempotent).

    out = max(min(in0, in1)*imm2 + s1, min(in0, in1))

    Base (1x) program comes from lower(); a hand-authored dual-chain
    2x_1p program is attached at perf slot +1 so the engine processes two
    bf16 elements per cycle when operands are packed 16-bit step-1 SBUF.
    """
    import concourse.dve_ops as dve_ops
    from concourse.dve_spec import Spec, Src0, Src1, C1, C2, maxx, minn, lower
    from concourse.dve_uop import (
        DveOpSpec,
        UopConfig,
        UopDpConfig,
        InpSel,
        OutSel,
        OutPath,
        AluInp,
        AluOp,
        DelayInp,
        Trigger,
    )

    name = "GAT_MLRS_ANT"
    for op in dve_ops.OPS:
        if op.name == name:
            return op

    def _ref(in0, in1, s0, s1, imm2):
        m = np.minimum(in0.astype(np.float32), in1.astype(np.float32))
        return np.maximum(m * imm2 + s1, m).astype(np.float32)

    _m = minn(Src0, Src1)
    spec = Spec(body=maxx(_m * C2 + C1, _m), reference=_ref)

    # ---- hand-authored 2x_1p uop: two parallel 4-op chains ------------
    # chain A (even elems): b0 MIN, b2 MUL, b4 ADD, b6 MAX
    # chain B (odd  elems): b1 MIN, b3 MUL, b5 ADD, b7 MAX
    # lanes: d0=SRC_1->mB, d1=SRC_0_HI->tA->qA->uA, d2=SRC_1_HI->tB->qB,
    #        d3=CONST_1(c1), d4=CONST_2(imm2), d5=mA
    def pd(k):
        return AluInp(int(AluInp.PREV_DELAY_0) + k)

    u2 = UopConfig()
    u2.enable_input(InpSel.SRC_0, 0)       # ALU path into block 0
    u2.enable_input(InpSel.SRC_1, 1)       # -> delay lane 0
    u2.enable_input(InpSel.SRC_0_HI, 2)    # -> delay lane 1
    u2.enable_input(InpSel.SRC_1_HI, 3)    # -> delay lane 2
    u2.enable_input(InpSel.CONST_1, 4)     # -> delay lane 3  (s1 slot)
    u2.enable_input(InpSel.CONST_2, 5)     # -> delay lane 4  (imm2 slot)
    dp = u2.datapath_config
    for b in dp:
        b.pass_through_delay(0, 1, 2, 3, 4, 5)
    dp[0].enable_alu(AluOp.MIN, AluInp.PREV_ALU_OUT, pd(0))        # mA
    dp[1].enable_alu(AluOp.MIN, pd(1), pd(2))                      # mB
    dp[1].enable_delay_from_src(DelayInp.PREV_ALU_OUT, 5)          # d5 <- mA
    dp[2].enable_alu(AluOp.MULTIPLY, pd(5), pd(4))                 # tA
    dp[2].enable_delay_from_src(DelayInp.PREV_ALU_OUT, 0)          # d0 <- mB
    dp[3].enable_alu(AluOp.MULTIPLY, pd(0), pd(4))                 # tB
    dp[3].enable_delay_from_src(DelayInp.PREV_ALU_OUT, 1)          # d1 <- tA
    dp[4].enable_alu(AluOp.ADD, pd(1), pd(3))                      # qA
    dp[4].enable_delay_from_src(DelayInp.PREV_ALU_OUT, 2)          # d2 <- tB
    dp[5].enable_alu(AluOp.ADD, pd(2), pd(3))                      # qB
    dp[5].enable_delay_from_src(DelayInp.PREV_ALU_OUT, 1)          # d1 <- qA
    dp[6].enable_alu(AluOp.MAX, pd(1), pd(5))                      # uA
    dp[6].enable_delay_from_src(DelayInp.PREV_ALU_OUT, 2)          # d2 <- qB
    dp[7].enable_alu(AluOp.MAX, pd(2), pd(0))                      # uB
    dp[7].enable_delay_from_src(DelayInp.PREV_ALU_OUT, 1)          # d1 <- uA
    u2.enable_output(OutSel.DELAY_1, OutPath.WR0_LO)               # uA (even)
    u2.enable_output(OutSel.ALU_OUT, OutPath.WR0_HI)               # uB (odd)
    u2.require_inp0 = 1
    u2.require_inp1 = 1
    u2.trigger = (Trigger.SRC_TENSOR_DONE, Trigger.NONE, Trigger.NONE)

    row = 1 + len(dve_ops.OPS)
    ver = "v3"
    s = DveOpSpec(
        name=name,
        opcode=row,
        uops=lower(spec, ver=ver),
        uops_2x=[u2],
        perf_max=1,
        rd1_en=True,
    )
    op = dve_ops.DveOp(name, spec, subdim=False, uops_sha={ver: s.sha(ver)})
    dve_ops.OPS.append(op)
    dve_ops._SUB_OPCODE_FOR_NAME[name] = row
    dve_ops.CUSTOM_DVE_SPECS[name] = op.spec
    dve_ops._COMPILE_CACHE[(name, ver)] = s
    return op


GAT_OP = _register_gat_op()


def build_nc():
    nc = bacc.Bacc("TRN2", target_bir_lowering=False)
    hT_d = nc.dram_tensor("hT", [F_IN, N], BF16, kind="ExternalInput")
    hsT_d = nc.dram_tensor("hsT", [F_IN, R], BF16, kind="ExternalInput")
    adj_d = nc.dram_tensor("adj", [R, N], FP8, kind="ExternalInput")
    wT_d = nc.dram_tensor("wT", [F_OUT, F_IN], F32, kind="ExternalInput")
    a_d = nc.dram_tensor("a", [2 * F_OUT, 1], F32, kind="ExternalInput")
    out_d = nc.dram_tensor("out", [R, N], BF16, kind="ExternalOutput")

    with tile.TileContext(nc) as tc:
        with (
            tc.tile_pool(name="persist", bufs=1) as persist,
            tc.tile_pool(name="hTp", bufs=8) as hTp,
            tc.tile_pool(name="psB", bufs=4, space="PSUM") as psB,
            tc.tile_pool(name="psS", bufs=1, space="PSUM") as psS,
            tc.tile_pool(name="adjp8", bufs=2) as adjp8,
            tc.tile_pool(name="adjp", bufs=4) as adjp,
            tc.tile_pool(name="ep", bufs=5) as ep,
            tc.tile_pool(name="small", bufs=6) as small,
        ):
            # --------- setup: tiny params first, then the bulk streams ------
            wT_sb = persist.tile([F_OUT, F_IN], F32)
            nc.scalar.dma_start(out=wT_sb, in_=wT_d[:, :])
            # a2[o, j] = a[j*64 + o]: a_src / a_dst as two columns
            a2 = persist.tile([F_OUT, 2], F32)
            a_t = a_d.tensor if hasattr(a_d, "tensor") else a_d
            nc.scalar.dma_start(
                out=a2, in_=bass.AP(tensor=a_t, offset=0, ap=[[1, F_OUT], [F_OUT, 2]])
            )
            hsT_sb = persist.tile([P, R], BF16)
            nc.sync.dma_start(out=hsT_sb, in_=hsT_d[:, :])
            hTs = []
            for g in range(8):
                hTc = hTp.tile([P, N // 8], BF16, tag="hTc")
                eng = nc.scalar if g % 2 == 0 else nc.sync
                eng.dma_start(
                    out=hTc, in_=hT_d[:, g * (N // 8):(g + 1) * (N // 8)]
                )
                hTs.append(hTc)
            # adj loads, all on the SWDGE ring (FIFO order = emission order):
            #  - tiles 0 and 7 ride as raw fp8 (cheap 1 MB loads; their fused
            #    op auto-falls back to the 1x program since in1 is 1-byte).
            #    Tile 0 lands first so the loop starts early; its slower 1x
            #    fused op runs while ACT is still idle. Tile 7 queues last.
            #  - tiles 1-6 use the inline fp8e5 -> bf16 upconvert so their
            #    fused op runs 2x. Only the FIRST cast carries a gate on the
            #    last hT chunk (tiny dummy write -> WAR dep); the rest queue
            #    behind it on the same FIFO ring. Without the gate the 2MB
            #    cast streams starve the small hT loads (SDMA engines round-
            #    robin packet-wise across queues) and setup stretches 4x.
            adjts = {}
            adjt0 = adjp8.tile([P, N], FP8, tag="adjt8")
            nc.gpsimd.dma_start(out=adjt0, in_=adj_d[0:P, :])
            adjts[0] = adjt0
            for t in range(1, RT - 1):
                adjt = adjp.tile([P, N], BF16, tag="adjt")
                if t == 1:
                    nc.vector.tensor_scalar(
                        out=adjt[:, 0:1], in0=hTs[7][:, 0:1], scalar1=0.0,
                        scalar2=None, op0=ALU.mult,
                    )
                nc.gpsimd.dma_start(out=adjt, in_=adj_d[t * P:(t + 1) * P, :])
                adjts[t] = adjt
            adjt7 = adjp8.tile([P, N], FP8, tag="adjt8")
            nc.gpsimd.dma_start(out=adjt7, in_=adj_d[(RT - 1) * P:RT * P, :])
            adjts[RT - 1] = adjt7

            # wa12[:, j] = w @ (a_src if j==0 else a_dst), one K=64 matmul
            ps_wa = psS.tile([P, 2], F32, tag="pswa")
            nc.tensor.matmul(ps_wa, lhsT=wT_sb, rhs=a2, start=True, stop=True)
            wa12 = persist.tile([P, 2], F32)
            nc.scalar.copy(wa12, ps_wa)

            # W2B[f, p] = C * wa2[f]  (stationary matrix for the BC2 matmuls)
            ones = persist.tile([P, P], BF16)
            nc.vector.memset(ones, 1.0)
            w2b = persist.tile([P, P], BF16)
            nc.vector.tensor_scalar(
                out=w2b, in0=ones, scalar1=wa12[:, 1:2], scalar2=C,
                op0=ALU.mult, op1=ALU.mult,
            )
            wa1c = persist.tile([P, 1], BF16)
            nc.vector.tensor_scalar(
                out=wa1c, in0=wa12[:, 0:1], scalar1=C, scalar2=None, op0=ALU.mult
            )

            # s1c[r, t] = C * s1[t*128 + r]  for this core's 8 row tiles
            ps_s1 = psS.tile([P, RT], F32)
            for t in range(RT):
                nc.tensor.matmul(
                    ps_s1[:, t:t + 1], lhsT=hsT_sb[:, t * P:(t + 1) * P],
                    rhs=wa1c, start=True, stop=True,
                )
            s1c = persist.tile([P, RT], F32)
            nc.scalar.copy(s1c, ps_s1)

            # BC2 chunks: psb = C*s2[j] broadcast over partitions.
            # Chunk 0 additionally yields the shift K ~= max_j s2 (a 512-col
            # sample max -- only rounding centering depends on it), then every
            # chunk is copied PSUM -> bc2p with the -K bias applied BEFORE the
            # bf16 rounding (ACT Identity-with-bias; a few ride DVE to
            # balance engine load).
            bc2p = persist.tile([P, N], BF16)
            s2maxc = persist.tile([P, 1], F32)
            sneg = persist.tile([P, 1], F32)
            c1c = persist.tile([P, RT], F32)
            for cg in range(16):
                psb = psB.tile([P, 512], F32, tag="psb")
                nc.tensor.matmul(
                    psb, lhsT=w2b,
                    rhs=hTs[cg // 2][:, (cg % 2) * 512:(cg % 2) * 512 + 512],
                    start=True, stop=True,
                )
                if cg == 0:
                    nc.vector.tensor_reduce(
                        s2maxc, psb, axis=mybir.AxisListType.X, op=ALU.max
                    )
                    nc.vector.tensor_scalar(
                        out=sneg, in0=s2maxc, scalar1=-1.0, scalar2=None,
                        op0=ALU.mult,
                    )
                    # c1[p, t] = -0.8 * (C*s1 + K)
                    nc.vector.tensor_scalar(
                        out=c1c, in0=s1c, scalar1=s2maxc[:, 0:1], scalar2=-0.8,
                        op0=ALU.add, op1=ALU.mult,
                    )
                sl = slice(cg * 512, (cg + 1) * 512)
                # all copies on ACT: the scalar engine is idle during setup
                # (first Exp can only start after fused(0)), so these are
                # free; DVE setup work would delay fused(0).
                nc.scalar.activation(
                    out=bc2p[:, sl], in_=psb, func=AF.Identity,
                    bias=sneg[:, 0:1], scale=1.0,
                )

            # ---------------- main loop over row tiles (sw-pipelined) -------
            # chain per tile:
            #   u = max(min(bc2p, adj)*0.2 + c1, min(bc2p, adj))  (DVE 2x)
            #   p = Exp(2^40 * u), S = rowsum                     (ACT)
            #   out = p * (1/S)                                   (DVE 4x)
            def emit_fused(t):
                et = ep.tile([P, N], BF16, tag="et")
                bi = nc.vector._custom_dve(
                    GAT_OP, out=et, in0=bc2p, in1=adjts[t],
                    s0=0.0, s1=c1c[:, t:t + 1], imm2=ALPHA,
                )
                bi.ins.perf_max = 1
                return et

            LOOKAHEAD = 2
            ets = {t: emit_fused(t) for t in range(min(LOOKAHEAD, RT))}
            for t in range(RT):
                if t + LOOKAHEAD < RT:
                    ets[t + LOOKAHEAD] = emit_fused(t + LOOKAHEAD)
                et = ets.pop(t)
                S = small.tile([P, 1], F32, tag="S")
                nc.scalar.activation(
                    out=et, in_=et, func=AF.Exp,
                    bias=0.0, scale=CI, accum_out=S,
                )
                rs = small.tile([P, 1], F32, tag="rs")
                nc.vector.reciprocal(rs, S)
                # scale + store in halves: each half streams out on its own
                # HWDGE ring as soon as it is scaled, halving the store tail
                # and splitting ring pressure.
                H = N // 2
                for hx in range(2):
                    sl = slice(hx * H, (hx + 1) * H)
                    nc.vector.tensor_scalar(
                        out=et[:, sl], in0=et[:, sl], scalar1=rs[:, 0:1],
                        scalar2=None, op0=ALU.mult,
                    )
                    eng = nc.sync if hx == 0 else nc.scalar
                    eng.dma_start(
                        out=out_d[t * P:(t + 1) * P, sl], in_=et[:, sl]
                    )

    nc.compile()
    return nc


def kernel(h, adj, w, a):
    global _CACHED_NC, LAST_RESULT
    h = np.ascontiguousarray(h, dtype=np.float32)
    adj = np.ascontiguousarray(adj, dtype=np.float32)
    w = np.ascontiguousarray(w, dtype=np.float32)
    a = np.ascontiguousarray(a, dtype=np.float32)

    if _CACHED_NC is None:
        _CACHED_NC = build_nc()
    nc = _CACHED_NC

    hT = np.ascontiguousarray(h.T.astype(ml_dtypes.bfloat16))
    wT = np.ascontiguousarray(w.T)
    in_maps = [
        {
            "hT": hT,
            "hsT": np.ascontiguousarray(hT[:, i * R:(i + 1) * R]),
            "adj": np.ascontiguousarray(
                adj[i * R:(i + 1) * R].astype(ml_dtypes.float8_e5m2)
            ),
            "wT": wT,
            "a": a,
        }
        for i in range(NCORES)
    ]
    res = run_bass_kernel_spmd(nc, in_maps, core_ids=list(range(NCORES)))
    LAST_RESULT = res
    return np.concatenate(
        [r["out"].astype(np.float32) for r in res.results], axis=0
    )


# revision 11
# speedup vs baseline: 1.1763x; 1.0715x over previous
"""GAT attention layer (EEGGraphAttentionLayer) for Trainium2, 8 NeuronCores.

reference math:
    Wh = h @ w                         # (8192, 64)
    e  = leaky_relu((Wh@a_src) + (Wh@a_dst).T, slope=0.2)   # (8192, 8192)
    att = where(adj > 0, e, -1e12)
    out = softmax(att, axis=1)

Sharding: rows of adj/out across 8 cores (1024 rows each); row softmax is
core-local. Each core recomputes the column-score vector s2 = h @ (w@a_dst)
(an N-vector) from the full h. h and w are host-transposed and h/adj
precision-reduced (bf16 / fp8e5 -- pure dtype casts); output bf16, upcast
on host (exact).

Math restructure (key to speed): with per-row shift M_i = s1_i + K
(K ~= max_j s2_j), max distributes over the shift so
    lrelu(z) - M = max(z - M, 0.2*z - M)
                 = max(m, 0.2*m - 0.8*C*M)   with m = C*(s2_j - K) row-free!
The fused per-element op is only FOUR ALU ops:
    u = max(min(bc2', adj)*0.2 + c1[p], min(bc2', adj))
with bc2'[j] = C*(s2_j - K) (bf16, shared by all row tiles) and
c1[p] = -0.8*C*(s1_p + K) per-partition. Masked entries ride the C-scale
min trick: |C*(s2-K)| <= ~5e-11 << 2^-16 (smallest positive fp8), so min
selects adj (<= -2^-16) when masked and exp(2^40 * 0.2*adj) underflows to
exactly 0. The shift K comes from one 512-column PSUM chunk (cheap, and any
K near max(s2) only affects rounding centering, not correctness).

Engine mapping per row tile [128, 8192]:
    fused op   DVE  custom GAT_MLRS_ANT with a hand-authored 2x_1p uop
               program (dual 4-op chains in the 8 ALU blocks) -> 2 elem/cyc.
               adj is upconverted fp8e5->bf16 *inside the load DMA* (SWDGE
               cast) so both tensor operands are 2-byte = 2x eligible,
               while HBM traffic stays 1 B/elem.
    exp+rowsum ACT  one Exp pass, scale=2^40, accum_out (1 elem/cyc)
    out*(1/S)  DVE  tensor_scalar, bf16 4x mode
HBM per core: 8 MB adj(fp8) + 16 MB out(bf16) + 2.25 MB h.
"""
import os
import sys

for _p in (
    "/opt/trn_rl_repo",
    "/root/.axon_site/_ro/trn_rl_repo",
):
    if os.path.isdir(_p) and _p not in sys.path:
        sys.path.append(_p)

import numpy as np
import ml_dtypes


def _install_profile_shim():
    """bass_utils' trace path imports antenv.axon_hooks, which this image
    lacks. Provide it (with the ctypes hook into libaxon if available) so a
    BASS_TRACE=1 run profiles instead of crashing. No-op on any failure."""
    import contextlib
    import ctypes
    import types

    if "antenv.axon_hooks" in sys.modules:
        return
    try:
        import antenv
    except ImportError:
        return

    def _make_hook(so_path):
        try:
            lib = ctypes.CDLL(so_path)
        except OSError:
            return None
        if not hasattr(lib, "axon_start_nrt_profile"):
            return None
        lib.axon_start_nrt_profile.argtypes = [
            ctypes.POINTER(ctypes.c_int64),
            ctypes.c_size_t,
        ]
        lib.axon_start_nrt_profile.restype = ctypes.c_int64
        lib.axon_stop_nrt_profile.argtypes = [ctypes.c_char_p]
        lib.axon_stop_nrt_profile.restype = ctypes.c_int64

        @contextlib.contextmanager
        def _hook(output_dir, device_ids):
            import jax

            jax.devices()
            if device_ids:
                ids = (ctypes.c_int64 * len(device_ids))(*device_ids)
                rc = lib.axon_start_nrt_profile(ids, len(device_ids))
            else:
                rc = lib.axon_start_nrt_profile(None, 0)
            if rc != 0:
                raise RuntimeError(f"axon_start_nrt_profile rc={rc}")
            try:
                yield
            finally:
                n = lib.axon_stop_nrt_profile(str(output_dir).encode())
                print(f"profile: {n} file(s) -> {output_dir}", file=sys.stderr)

        return _hook

    hook = [_make_hook("/opt/axon/libaxon_pjrt.so")]
    mod = types.ModuleType("antenv.axon_hooks")
    mod.set_axon_ntff_profile_hook = lambda h: hook.__setitem__(0, h)
    mod.get_axon_ntff_profile_hook = lambda: hook[0]
    sys.modules["antenv.axon_hooks"] = mod
    antenv.axon_hooks = mod


try:
    _install_profile_shim()
except Exception:
    pass

import concourse.bacc as bacc
import concourse.tile as tile
import concourse.bass as bass
from concourse import mybir
from concourse.bass_utils import run_bass_kernel_spmd

N, F_IN, F_OUT = 8192, 128, 64
NCORES = 8
R = N // NCORES          # rows per core (1024)
P = 128                  # SBUF partitions
RT = R // P              # row tiles per core (8)
C = 2.0 ** -40           # exact scale-down of scores
CI = 2.0 ** 40
ALPHA = 0.2              # leaky relu negative slope
F32 = mybir.dt.float32
BF16 = mybir.dt.bfloat16
FP8 = mybir.dt.float8e5
AF = mybir.ActivationFunctionType
ALU = mybir.AluOpType

_CACHED_NC = None
LAST_RESULT = None       # BassKernelResults of the most recent run (for tests)


def _register_gat_op():
    """Register the fused mask+lrelu custom DVE op (idempotent).

    out = max(min(in0, in1)*imm2 + s1, min(in0, in1))

    Base (1x) program comes from lower(); a hand-authored dual-chain
    2x_1p program is attached at perf slot +1 so the engine processes two
    bf16 elements per cycle when operands are packed 16-bit step-1 SBUF.
    """
    import concourse.dve_ops as dve_ops
    from concourse.dve_spec import Spec, Src0, Src1, C1, C2, maxx, minn, lower
    from concourse.dve_uop import (
        DveOpSpec,
        UopConfig,
        UopDpConfig,
        InpSel,
        OutSel,
        OutPath,
        AluInp,
        AluOp,
        DelayInp,
        Trigger,
    )

    name = "GAT_MLRS_ANT"
    for op in dve_ops.OPS:
        if op.name == name:
            return op

    def _ref(in0, in1, s0, s1, imm2):
        m = np.minimum(in0.astype(np.float32), in1.astype(np.float32))
        return np.maximum(m * imm2 + s1, m).astype(np.float32)

    _m = minn(Src0, Src1)
    spec = Spec(body=maxx(_m * C2 + C1, _m), reference=_ref)

    # ---- hand-authored 2x_1p uop: two parallel 4-op chains ------------
    # chain A (even elems): b0 MIN, b2 MUL, b4 ADD, b6 MAX
    # chain B (odd  elems): b1 MIN, b3 MUL, b5 ADD, b7 MAX
    # lanes: d0=SRC_1->mB, d1=SRC_0_HI->tA->qA->uA, d2=SRC_1_HI->tB->qB,
    #        d3=CONST_1(c1), d4=CONST_2(imm2), d5=mA
    def pd(k):
        return AluInp(int(AluInp.PREV_DELAY_0) + k)

    u2 = UopConfig()
    u2.enable_input(InpSel.SRC_0, 0)       # ALU path into block 0
    u2.enable_input(InpSel.SRC_1, 1)       # -> delay lane 0
    u2.enable_input(InpSel.SRC_0_HI, 2)    # -> delay lane 1
    u2.enable_input(InpSel.SRC_1_HI, 3)    # -> delay lane 2
    u2.enable_input(InpSel.CONST_1, 4)     # -> delay lane 3  (s1 slot)
    u2.enable_input(InpSel.CONST_2, 5)     # -> delay lane 4  (imm2 slot)
    dp = u2.datapath_config
    for b in dp:
        b.pass_through_delay(0, 1, 2, 3, 4, 5)
    dp[0].enable_alu(AluOp.MIN, AluInp.PREV_ALU_OUT, pd(0))        # mA
    dp[1].enable_alu(AluOp.MIN, pd(1), pd(2))                      # mB
    dp[1].enable_delay_from_src(DelayInp.PREV_ALU_OUT, 5)          # d5 <- mA
    dp[2].enable_alu(AluOp.MULTIPLY, pd(5), pd(4))                 # tA
    dp[2].enable_delay_from_src(DelayInp.PREV_ALU_OUT, 0)          # d0 <- mB
    dp[3].enable_alu(AluOp.MULTIPLY, pd(0), pd(4))                 # tB
    dp[3].enable_delay_from_src(DelayInp.PREV_ALU_OUT, 1)          # d1 <- tA
    dp[4].enable_alu(AluOp.ADD, pd(1), pd(3))                      # qA
    dp[4].enable_delay_from_src(DelayInp.PREV_ALU_OUT, 2)          # d2 <- tB
    dp[5].enable_alu(AluOp.ADD, pd(2), pd(3))                      # qB
    dp[5].enable_delay_from_src(DelayInp.PREV_ALU_OUT, 1)          # d1 <- qA
    dp[6].enable_alu(AluOp.MAX, pd(1), pd(5))                      # uA
    dp[6].enable_delay_from_src(DelayInp.PREV_ALU_OUT, 2)          # d2 <- qB
    dp[7].enable_alu(AluOp.MAX, pd(2), pd(0))                      # uB
    dp[7].enable_delay_from_src(DelayInp.PREV_ALU_OUT, 1)          # d1 <- uA
    u2.enable_output(OutSel.DELAY_1, OutPath.WR0_LO)               # uA (even)
    u2.enable_output(OutSel.ALU_OUT, OutPath.WR0_HI)               # uB (odd)
    u2.require_inp0 = 1
    u2.require_inp1 = 1
    u2.trigger = (Trigger.SRC_TENSOR_DONE, Trigger.NONE, Trigger.NONE)

    row = 1 + len(dve_ops.OPS)
    ver = "v3"
    s = DveOpSpec(
        name=name,
        opcode=row,
        uops=lower(spec, ver=ver),
        uops_2x=[u2],
        perf_max=1,
        rd1_en=True,
    )
    op = dve_ops.DveOp(name, spec, subdim=False, uops_sha={ver: s.sha(ver)})
    dve_ops.OPS.append(op)
    dve_ops._SUB_OPCODE_FOR_NAME[name] = row
    dve_ops.CUSTOM_DVE_SPECS[name] = op.spec
    dve_ops._COMPILE_CACHE[(name, ver)] = s
    return op


GAT_OP = _register_gat_op()


def build_nc():
    nc = bacc.Bacc("TRN2", target_bir_lowering=False)
    hT_d = nc.dram_tensor("hT", [F_IN, N], BF16, kind="ExternalInput")
    hsT_d = nc.dram_tensor("hsT", [F_IN, R], BF16, kind="ExternalInput")
    adj_d = nc.dram_tensor("adj", [R, N], FP8, kind="ExternalInput")
    wT_d = nc.dram_tensor("wT", [F_OUT, F_IN], F32, kind="ExternalInput")
    a_d = nc.dram_tensor("a", [2 * F_OUT, 1], F32, kind="ExternalInput")
    out_d = nc.dram_tensor("out", [R, N], BF16, kind="ExternalOutput")

    with tile.TileContext(nc) as tc:
        with (
            tc.tile_pool(name="persist", bufs=1) as persist,
            tc.tile_pool(name="hTp", bufs=8) as hTp,
            tc.tile_pool(name="psB", bufs=4, space="PSUM") as psB,
            tc.tile_pool(name="psS", bufs=1, space="PSUM") as psS,
            tc.tile_pool(name="adjp8", bufs=2) as adjp8,
            tc.tile_pool(name="adjp", bufs=4) as adjp,
            tc.tile_pool(name="ep", bufs=5) as ep,
            tc.tile_pool(name="small", bufs=6) as small,
        ):
            # --------- setup: tiny params first, then the bulk streams ------
            wT_sb = persist.tile([F_OUT, F_IN], F32)
            nc.scalar.dma_start(out=wT_sb, in_=wT_d[:, :])
            # a2[o, j] = a[j*64 + o]: a_src / a_dst as two columns
            a2 = persist.tile([F_OUT, 2], F32)
            a_t = a_d.tensor if hasattr(a_d, "tensor") else a_d
            nc.scalar.dma_start(
                out=a2, in_=bass.AP(tensor=a_t, offset=0, ap=[[1, F_OUT], [F_OUT, 2]])
            )
            hsT_sb = persist.tile([P, R], BF16)
            nc.sync.dma_start(out=hsT_sb, in_=hsT_d[:, :])
            hTs = []
            for g in range(8):
                hTc = hTp.tile([P, N // 8], BF16, tag="hTc")
                eng = nc.scalar if g % 2 == 0 else nc.sync
                eng.dma_start(
                    out=hTc, in_=hT_d[:, g * (N // 8):(g + 1) * (N // 8)]
                )
                hTs.append(hTc)
            # adj tiles. All adj DMAs ride the SWDGE ring (FIFO order =
            # emission order) and are collectively gated behind most of the
            # hT stream (one dummy write on the first tile -> WAR dep on its
            # DMA; the rest queue behind it). Ungated, the adj streams
            # starve the small hT loads (SDMA engines round-robin packet-
            # wise across queues) and the whole setup stretches ~4x.
            #  - tiles 0 and 7 ride as raw fp8 (cheap 1 MB loads; their
            #    fused op auto-falls back to the 1x program since in1 is
            #    1-byte). Tile 0 goes first so the loop starts early; its
            #    slower 1x fused op runs while ACT is still idle.
            #  - tiles 1-6 use the inline fp8e5 -> bf16 upconvert so their
            #    fused op runs 2x, with HBM traffic still 1 B/elem.
            # NOTE: the dummy gate is NOT emitted here -- the DVE queue is
            # in-order and a wait here would stall wa1c/w2b and through them
            # the whole PE pipeline (observed +11us). It is emitted after
            # c1c below, when the DVE has no setup work left.
            adjts = {}
            adjt0 = adjp8.tile([P, N], FP8, tag="adjt8")
            adjts[0] = adjt0

            def emit_adj_dmas():
                nc.gpsimd.dma_start(out=adjt0, in_=adj_d[0:P, :])
                for t in range(1, RT - 1):
                    adjt = adjp.tile([P, N], BF16, tag="adjt", name=f"adjt{t}")
                    nc.gpsimd.dma_start(
                        out=adjt, in_=adj_d[t * P:(t + 1) * P, :]
                    )
                    adjts[t] = adjt
                adjt7 = adjp8.tile([P, N], FP8, tag="adjt8", name="adjt7")
                nc.gpsimd.dma_start(
                    out=adjt7, in_=adj_d[(RT - 1) * P:RT * P, :]
                )
                adjts[RT - 1] = adjt7

            # wa12[:, j] = w @ (a_src if j==0 else a_dst), one K=64 matmul
            ps_wa = psS.tile([P, 2], F32, tag="pswa")
            nc.tensor.matmul(ps_wa, lhsT=wT_sb, rhs=a2, start=True, stop=True)
            wa12 = persist.tile([P, 2], F32)
            nc.scalar.copy(wa12, ps_wa)

            # W2B[f, p] = C * wa2[f]  (stationary matrix for the BC2 matmuls)
            ones = persist.tile([P, P], BF16)
            nc.vector.memset(ones, 1.0)
            w2b = persist.tile([P, P], BF16)
            nc.vector.tensor_scalar(
                out=w2b, in0=ones, scalar1=wa12[:, 1:2], scalar2=C,
                op0=ALU.mult, op1=ALU.mult,
            )
            wa1c = persist.tile([P, 1], BF16)
            nc.vector.tensor_scalar(
                out=wa1c, in0=wa12[:, 0:1], scalar1=C, scalar2=None, op0=ALU.mult
            )

            # s1c[r, t] = C * s1[t*128 + r]  for this core's 8 row tiles
            ps_s1 = psS.tile([P, RT], F32)
            for t in range(RT):
                nc.tensor.matmul(
                    ps_s1[:, t:t + 1], lhsT=hsT_sb[:, t * P:(t + 1) * P],
                    rhs=wa1c, start=True, stop=True,
                )
            s1c = persist.tile([P, RT], F32)
            nc.scalar.copy(s1c, ps_s1)

            # BC2 chunks: psb = C*s2[j] broadcast over partitions.
            # Chunk 0 additionally yields the shift K ~= max_j s2 (a 512-col
            # sample max -- only rounding centering depends on it), then every
            # chunk is copied PSUM -> bc2p with the -K bias applied BEFORE the
            # bf16 rounding (ACT Identity-with-bias; a few ride DVE to
            # balance engine load).
            bc2p = persist.tile([P, N], BF16)
            s2maxc = persist.tile([P, 1], F32)
            sneg = persist.tile([P, 1], F32)
            c1c = persist.tile([P, RT], F32)
            for cg in range(16):
                psb = psB.tile([P, 512], F32, tag="psb")
                nc.tensor.matmul(
                    psb, lhsT=w2b,
                    rhs=hTs[cg // 2][:, (cg % 2) * 512:(cg % 2) * 512 + 512],
                    start=True, stop=True,
                )
                if cg == 0:
                    nc.vector.tensor_reduce(
                        s2maxc, psb, axis=mybir.AxisListType.X, op=ALU.max
                    )
                    nc.vector.tensor_scalar(
                        out=sneg, in0=s2maxc, scalar1=-1.0, scalar2=None,
                        op0=ALU.mult,
                    )
                    # c1[p, t] = -0.8 * (C*s1 + K)
                    nc.vector.tensor_scalar(
                        out=c1c, in0=s1c, scalar1=s2maxc[:, 0:1], scalar2=-0.8,
                        op0=ALU.add, op1=ALU.mult,
                    )
                sl = slice(cg * 512, (cg + 1) * 512)
                # all copies on ACT: the scalar engine is idle during setup
                # (first Exp can only start after fused(0)), so these are
                # free; DVE setup work would delay fused(0).
                nc.scalar.activation(
                    out=bc2p[:, sl], in_=psb, func=AF.Identity,
                    bias=sneg[:, 0:1], scale=1.0,
                )

            # The adj gate: a tiny DVE write into the first adj tile that
            # waits on the 6th hT chunk. Its WAR dep holds back the whole
            # FIFO adj ring until hT has (mostly) landed. Emitted HERE so
            # the only DVE work behind it is the fused ops, which cannot
            # start earlier anyway.
            nc.vector.tensor_scalar(
                out=adjt0[:, 0:1], in0=hTs[5][:, 0:1], scalar1=0.0,
                scalar2=None, op0=ALU.mult,
            )
            emit_adj_dmas()

            # ---------------- main loop over row tiles (sw-pipelined) -------
            # chain per tile:
            #   u = max(min(bc2p, adj)*0.2 + c1, min(bc2p, adj))  (DVE 2x)
            #   p = Exp(2^40 * u), S = rowsum                     (ACT)
            #   out = p * (1/S)                                   (DVE 4x)
            H = N // 2

            def emit_fused(t):
                # tile 0 is emitted as two halves: its first half only needs
                # the first 8 bc2p chunks + the (1MB, early) fp8 adj tile,
                # so the exp pipeline starts ~6us sooner.
                et = ep.tile([P, N], BF16, tag="et", name=f"et{t}")
                nsplit = 2 if t == 0 else 1
                w = N // nsplit
                for hx in range(nsplit):
                    sl = slice(hx * w, (hx + 1) * w)
                    bi = nc.vector._custom_dve(
                        GAT_OP, out=et[:, sl], in0=bc2p[:, sl],
                        in1=adjts[t][:, sl],
                        s0=0.0, s1=c1c[:, t:t + 1], imm2=ALPHA,
                    )
                    bi.ins.perf_max = 1
                return et

            LOOKAHEAD = 2
            ets = {t: emit_fused(t) for t in range(min(LOOKAHEAD, RT))}
            for t in range(RT):
                if t + LOOKAHEAD < RT:
                    ets[t + LOOKAHEAD] = emit_fused(t + LOOKAHEAD)
                et = ets.pop(t)
                if t == 0:
                    S2 = small.tile([P, 2], F32, tag="S2")
                    for hx in range(2):
                        sl = slice(hx * H, (hx + 1) * H)
                        nc.scalar.activation(
                            out=et[:, sl], in_=et[:, sl], func=AF.Exp,
                            bias=0.0, scale=CI, accum_out=S2[:, hx:hx + 1],
                        )
                    S = small.tile([P, 1], F32, tag="S")
                    nc.vector.tensor_scalar(
                        out=S, in0=S2[:, 0:1], scalar1=S2[:, 1:2],
                        scalar2=None, op0=ALU.add,
                    )
                else:
                    S = small.tile([P, 1], F32, tag="S")
                    nc.scalar.activation(
                        out=et, in_=et, func=AF.Exp,
                        bias=0.0, scale=CI, accum_out=S,
                    )
                rs = small.tile([P, 1], F32, tag="rs")
                nc.vector.reciprocal(rs, S)
                # scale + store in halves: each half streams out on its own
                # HWDGE ring as soon as it is scaled, halving the store tail
                # and splitting ring pressure.
                for hx in range(2):
                    sl = slice(hx * H, (hx + 1) * H)
                    nc.vector.tensor_scalar(
                        out=et[:, sl], in0=et[:, sl], scalar1=rs[:, 0:1],
                        scalar2=None, op0=ALU.mult,
                    )
                    eng = nc.sync if hx == 0 else nc.scalar
                    eng.dma_start(
                        out=out_d[t * P:(t + 1) * P, sl], in_=et[:, sl]
                    )

    nc.compile()
    return nc


def kernel(h, adj, w, a):
    global _CACHED_NC, LAST_RESULT
    h = np.ascontiguousarray(h, dtype=np.float32)
    adj = np.ascontiguousarray(adj, dtype=np.float32)
    w = np.ascontiguousarray(w, dtype=np.float32)
    a = np.ascontiguousarray(a, dtype=np.float32)

    if _CACHED_NC is None:
        _CACHED_NC = build_nc()
    nc = _CACHED_NC

    hT = np.ascontiguousarray(h.T.astype(ml_dtypes.bfloat16))
    wT = np.ascontiguousarray(w.T)
    in_maps = [
        {
            "hT": hT,
            "hsT": np.ascontiguousarray(hT[:, i * R:(i + 1) * R]),
            "adj": np.ascontiguousarray(
                adj[i * R:(i + 1) * R].astype(ml_dtypes.float8_e5m2)
            ),
            "wT": wT,
            "a": a,
        }
        for i in range(NCORES)
    ]
    res = run_bass_kernel_spmd(nc, in_maps, core_ids=list(range(NCORES)))
    LAST_RESULT = res
    return np.concatenate(
        [r["out"].astype(np.float32) for r in res.results], axis=0
    )
